# revision 32
# baseline (speedup 1.0000x reference)
"""Trainium2 Bass kernel for C = tril(A @ B), A/B lower-triangular 4096x4096 fp32.

Load-balanced slot design (SPMD, 8 cores = 2 teams x 4 rows). The 144 output
tiles (row-block r, 512-col band j; r >= 4j) have k-extents e = r+1-4j that a
uniform program must round up; the classic row/col split costs 1000
column-units per core. This kernel instead runs 18 shape-matched SLOTS per
core totaling 884 units (the optimum for 4-k-block-quantized shapes): each
slot is an accumulation group [128 x 512] with a fixed k-depth E, reading a
fixed window of one of 5 shared B REGIONS. Which (band, rows) a slot computes
varies per core purely through host-side packing: team A (cores 0-3) covers
bands {0,2,4,6} (+ b2 tail), team B (cores 4-7) bands {1,2,3,5,6,7}; within a
team, core g takes row base+g of each slot's quad. Every core runs the
identical instruction stream; all over-computed terms multiply structural
zeros, so results are exact.

Regions (per-core B traffic 96 k-blocks, fp8): R0 32kb, R1 24kb, R2 20kb,
R3 16kb, R4 4kb; per-team content starts (kappa) chosen so slot windows
cover each hosted tile's true k-range [4j, r].

Operand dtypes: A^T fp16 (stationary), B fp8 e3m4 (moving) - same 1 col/cycle
PE rate, half the B bytes (kernel is near the per-core ~330GB/s DMA
roofline). PSUM accumulates fp32; outputs evict via DVE to fp16. B's fp8
quantization dominates the error: ~1.34e-2 relative on the fixed test inputs
(gate 2e-2).

Schedule: regions sequential, slots within a region small-to-large so the PE
starts on minimal data and the exit tail ends on the tiny 4-deep slot; two
extra tiny warm-up slots (regions 2/3) interleave right after the first so
the pipe fills while region 0's bulk streams in. All B chunks and A slabs are
SBUF-resident (~14MB); loads issue upfront in first-need order (B on sync
HWDGE, A on scalar HWDGE, evictions on gpsimd SWDGE except the last regions
on sync).

Further trims: 6 of the 18 A slabs alias sub-windows of earlier slabs (A
slabs depend only on (row, k-window), which coincide on both teams), cutting
A traffic 248 -> 188 k-blocks; 6 dummy matmuls gated only on slot 0's A slab
burn the first-B-chunk DMA wait so the tensor engine's ~1.1 -> 2.4GHz clock
ramp (~3-4us of sustained work) completes before the real stream starts.

Measured: ~68-70us HW exec (best 67.2us, +-2.5us run-to-run HW noise) at
1.343e-2 relative error vs 76-80us / 2.9e-4 for the 4x2 row/col-band fp16
predecessor. Breakdown: ~4us DMA-latency startup, ~51-55us matmul stream
(884 units x ~57ns), ~11.8us tail of which ~7us is the fixed NEFF
semaphore-sweep epilogue.
"""

import numpy as np

N = 4096
P = 128
NCORES = 8
CW = 512
KC = 4                   # k-blocks per B chunk

A_DT_NAME = "float16"
B_DT_NAME = "float8e3"

# regions (in processing order): (size_kb, kappa_teamA, kappa_teamB, bandA, bandB)
REGIONS = [
    (32, 0, 4, 0, 1),
    (24, 0, 8, 0, 2),
    (20, 8, 12, 2, 3),
    (16, 16, 20, 4, 5),
    (12, 16, 20, 4, 6),
    (4, 24, 28, 6, 7),
]

# slots: (E_kb, region, rowbaseA, rowbaseB)
# team A: band REGIONS[r][3], row rowbaseA+g ; team B: band REGIONS[r][4]
SLOT_DEFS = [
    (4, 0, 0, 4),     # b0 r0-3   | b1 r4-7
    (8, 0, 4, 8),     # b0 r4-7   | b1 r8-11
    (12, 0, 8, 12),   # b0 r8-11  | b1 r12-15
    (16, 0, 12, 16),  # b0 r12-15 | b1 r16-19
    (20, 0, 16, 20),  # b0 r16-19 | b1 r20-23
    (28, 0, 24, 24),  # b0 r24-27 | b1 r24-27
    (32, 0, 28, 28),  # b0 r28-31 | b1 r28-31
    (24, 1, 20, 28),  # b0 r20-23 | b2 r28-31
    (4, 2, 8, 12),    # b2 r8-11  | b3 r12-15
    (8, 2, 12, 16),   # b2 r12-15 | b3 r16-19
    (12, 2, 16, 20),  # b2 r16-19 | b3 r20-23
    (16, 2, 20, 24),  # b2 r20-23 | b3 r24-27
    (20, 2, 24, 28),  # b2 r24-27 | b3 r28-31
    (4, 3, 16, 20),   # b4 r16-19 | b5 r20-23
    (8, 3, 20, 24),   # b4 r20-23 | b5 r24-27
    (16, 3, 28, 28),  # b4 r28-31 | b5 r28-31
    (12, 4, 24, 28),  # b4 r24-27 | b6 r28-31
    (4, 5, 24, 28),   # b6 r24-27 | b7 r28-31
]

NSLOT = len(SLOT_DEFS)

# A-slab aliasing: slot -> (base_slot, k-offset). A slabs depend only on
# (row, k-window); these slots' windows are sub-windows of an earlier slot's
# slab at the same offset on BOTH teams, so they read the base slab instead
# of loading their own (saves 68 of 248 k-blocks of A traffic).
# (s8/s13 qualify too but stay as their own tiny loads: they are the early
# warm-up slots and must not wait on their base slots' big slabs)
A_ALIAS = {9: (3, 8), 10: (4, 8),
           14: (11, 8), 15: (6, 16), 16: (12, 8), 17: (12, 16)}

# slabs stored fp8 e3m4 (the big ones; ~66% of MACs incl. aliases). With B
# also fp8, these slots' error is sqrt(2) of the B-only figure; the blend
# measures 1.742e-2 on the fixed inputs (gate 2e-2).
A_FP8 = {5, 6, 7, 11, 12}

A_OFF = [0] * NSLOT
_o16 = _o8 = 0
for _s, (_e, _r, _a, _b) in enumerate(SLOT_DEFS):
    if _s in A_ALIAS:
        continue
    if _s in A_FP8:
        A_OFF[_s] = _o8
        _o8 += _e
    else:
        A_OFF[_s] = _o16
        _o16 += _e
A_TOT16 = _o16                        # 68 k-blocks fp16
A_TOT8 = _o8                          # 120 k-blocks fp8

# B chunk list: (region, cc) in load order
B_CHUNKS = [(r, cc) for r in range(len(REGIONS))
            for cc in range((REGIONS[r][0] + KC - 1) // KC)]
B_CI = {rc: i for i, rc in enumerate(B_CHUNKS)}
NCHUNK = len(B_CHUNKS)                # 24

_cached = {}


def _slot_item(s, team, g):
    """(band j, row-block r, kappa) computed by slot s on core (team, g)."""
    e, reg, rbA, rbB = SLOT_DEFS[s]
    size, kA, kB, bA, bB = REGIONS[reg]
    if team == 0:
        return bA, rbA + g, kA
    return bB, rbB + g, kB


def _build(a_dt_name, b_dt_name):
    import concourse.mybir as mybir
    import concourse.tile as tile
    from concourse import bacc

    a_dt = getattr(mybir.dt, a_dt_name)
    b_dt = getattr(mybir.dt, b_dt_name)

    nc = bacc.Bacc("TRN2", target_bir_lowering=False, debug=False,
                   num_devices=NCORES)
    at_d = nc.dram_tensor("at", [P, A_TOT16 * P], a_dt,
                          kind="ExternalInput").ap()
    at8_d = nc.dram_tensor("at8", [P, A_TOT8 * P], b_dt,
                           kind="ExternalInput").ap()
    b_d = nc.dram_tensor("b", [NCHUNK * P, KC * CW], b_dt,
                         kind="ExternalInput").ap()
    o_d = nc.dram_tensor("o", [NSLOT, P, CW], mybir.dt.float16,
                         kind="ExternalOutput").ap()

    # per-region slot order: ascending E so the stream starts tiny
    reg_slots = [[] for _ in REGIONS]
    for s, (e, reg, _, _) in enumerate(SLOT_DEFS):
        reg_slots[reg].append(s)
    for rs in reg_slots:
        rs.sort(key=lambda s: SLOT_DEFS[s][0])

    with tile.TileContext(nc) as tc:
        with (
            tc.tile_pool(name="atp", bufs=1) as atp,
            tc.tile_pool(name="bp", bufs=1) as bp,
            tc.tile_pool(name="pp", bufs=4, space="PSUM") as pp,
            tc.tile_pool(name="sp", bufs=3) as sp,
        ):
            at_sb = {}
            chunks = {}
            loaded_a = set()
            loaded_b = set()

            def load_at(s):
                if s in A_ALIAS:
                    return load_at(A_ALIAS[s][0])
                if s in loaded_a:
                    return
                loaded_a.add(s)
                e = SLOT_DEFS[s][0]
                dt = b_dt if s in A_FP8 else a_dt
                src = at8_d if s in A_FP8 else at_d
                a = atp.tile([P, e, P], dt, tag=f"at{s}", name=f"at{s}")
                o0 = A_OFF[s] * P
                nc.scalar.dma_start(a[:], src[:, o0:o0 + e * P])
                at_sb[s] = a

            def load_chunk(reg, cc):
                if (reg, cc) in loaded_b:
                    return
                loaded_b.add((reg, cc))
                ci = B_CI[(reg, cc)]
                w = min(KC, REGIONS[reg][0] - KC * cc)
                bch = bp.tile([P, w, CW], b_dt, tag=f"b{ci}", name=f"b{ci}")
                nc.sync.dma_start(bch[:], b_d[ci * P:(ci + 1) * P, :w * CW])
                chunks[(reg, cc)] = bch

            def b_rhs(reg, k, w):
                cc, q = k // KC, k % KC
                return chunks[(reg, cc)][:, q, :w]

            def at_lhs(s, k):
                if s in A_ALIAS:
                    base, off = A_ALIAS[s]
                    return at_sb[base][:, off + k, :]
                return at_sb[s][:, k, :]

            def evict(s, ps, reg):
                st = sp.tile([P, CW], mybir.dt.float16, tag="st",
                             name=f"st{s}")
                nc.vector.tensor_copy(st[:], ps[:])
                eng = nc.sync if reg >= 4 else nc.gpsimd
                eng.dma_start(o_d[s, :, :], st[:])

            # processing order: tiny warm-up slots from other regions fill
            # the pipe while region 0's bulk data streams in; loads are
            # issued strictly in first-need order (JIT, 2 slots ahead)
            order = [0, 8, 13, 1, 2, 3, 4, 5, 6, 7, 9, 10, 11, 12,
                     14, 15, 16, 17]

            def ensure_loaded(s):
                e, reg = SLOT_DEFS[s][0], SLOT_DEFS[s][1]
                load_at(s)
                for cc in range((e + KC - 1) // KC):
                    load_chunk(reg, cc)

            for s in order:
                ensure_loaded(s)

            # PE clock-ramp warm-up: the tensor engine runs at ~1.1GHz until
            # ~3-4us of sustained work. Dummy matmuls gated only on slot 0's
            # A slab (which lands before the first B chunk) burn the B-DMA
            # wait window so the real stream starts at full clock. Results
            # go to a throwaway PSUM tile.
            dps = pp.tile([P, CW], mybir.dt.float32, tag="dps", name="dps")

            def dummies(n):
                for _ in range(n):
                    nc.tensor.matmul(
                        dps[:], lhsT=at_sb[0][:, 0, :],
                        rhs=at_sb[0][:, 0:KC, :], start=True, stop=True)

            dummies(6)
            for i, s in enumerate(order):
                e, reg = SLOT_DEFS[s][0], SLOT_DEFS[s][1]
                ps = pp.tile([P, CW], mybir.dt.float32, tag="ps",
                             name=f"ps{s}")
                for k in range(e):
                    w = min(CW, P * (k + 1))
                    nc.tensor.matmul(
                        ps[:, :w], lhsT=at_lhs(s, k), rhs=b_rhs(reg, k, w),
                        start=(k == 0), stop=(k == e - 1))
                evict(s, ps, reg)


    nc.compile()
    return nc


def _get_nc(a_dt_name, b_dt_name):
    key = (a_dt_name, b_dt_name, "v2")
    if key not in _cached:
        _cached[key] = _build(a_dt_name, b_dt_name)
    return _cached[key]


def _np_dt(dt_name):
    if dt_name == "float16":
        return np.float16
    if dt_name == "bfloat16":
        import ml_dtypes
        return ml_dtypes.bfloat16
    if dt_name == "float8e4":
        import ml_dtypes
        return ml_dtypes.float8_e4m3
    if dt_name == "float8e3":
        import ml_dtypes
        return ml_dtypes.float8_e3m4
    return np.float32


def _pack_at_core(A16, A8, team, g):
    """Two packs [128, A_TOT{16,8}*128]: slot s cols = A[rows of block r,
    128*kappa ...] laid out (p, k, m); fp8 slabs in the second array."""
    out16 = np.zeros((P, A_TOT16 * P), dtype=A16.dtype)
    out8 = np.zeros((P, A_TOT8 * P), dtype=A8.dtype)
    for s, (e, reg, rbA, rbB) in enumerate(SLOT_DEFS):
        if s in A_ALIAS:
            continue
        src, out = (A8, out8) if s in A_FP8 else (A16, out16)
        j, r, kap = _slot_item(s, team, g)
        k0 = kap * P
        k1 = min((kap + e) * P, N)
        ww = k1 - k0
        blockT = src[r * P:(r + 1) * P, k0:k1].T          # [ww, 128]
        arr = np.zeros((e * P, P), dtype=src.dtype)
        arr[:ww] = blockT
        arr = arr.reshape(e, P, P).transpose(1, 0, 2).reshape(P, e * P)
        out[:, A_OFF[s] * P:(A_OFF[s] + e) * P] = arr
    return out16, out8


def _pack_b_core(B8, team):
    """[NCHUNK*128, 2048]: region contents, 4-kb chunks, partition-major."""
    out = np.zeros((NCHUNK * P, KC * CW), dtype=B8.dtype)
    for reg, (size, kA, kB, bA, bB) in enumerate(REGIONS):
        kap = kA if team == 0 else kB
        j = bA if team == 0 else bB
        k0 = kap * P
        k1 = min((kap + size) * P, N)
        content = np.zeros((size * P, CW), dtype=B8.dtype)
        content[:k1 - k0] = B8[k0:k1, j * CW:(j + 1) * CW]
        c3 = content.reshape(size, P, CW)
        for cc in range((size + KC - 1) // KC):
            w = min(KC, size - KC * cc)
            ci = B_CI[(reg, cc)]
            out[ci * P:(ci + 1) * P, :w * CW] = (
                c3[cc * KC:cc * KC + w].transpose(1, 0, 2).reshape(P, w * CW))
    return out


def kernel(A, B, a_dt_name=A_DT_NAME, b_dt_name=B_DT_NAME, trace=False,
           **_ignored):
    from concourse.bass_utils import run_bass_kernel_spmd

    A = np.ascontiguousarray(np.asarray(A, dtype=np.float32))
    B = np.ascontiguousarray(np.asarray(B, dtype=np.float32))

    nc = _get_nc(a_dt_name, b_dt_name)
    A16 = A.astype(_np_dt(a_dt_name))
    A8 = A.astype(_np_dt(b_dt_name))
    B8 = B.astype(_np_dt(b_dt_name))
    b_packs = [_pack_b_core(B8, team) for team in range(2)]
    in_maps = []
    for c in range(NCORES):
        p16, p8 = _pack_at_core(A16, A8, c // 4, c % 4)
        in_maps.append({"at": p16, "at8": p8, "b": b_packs[c // 4]})

    res = None
    for attempt in range(3):
        try:
            res = run_bass_kernel_spmd(nc, in_maps,
                                       core_ids=list(range(NCORES)),
                                       trace=trace)
            break
        except Exception:
            if attempt == 2:
                raise
            import time
            time.sleep(2)
    C = np.zeros((N, N), dtype=np.float32)
    for c in range(NCORES):
        team, g = c // 4, c % 4
        o = np.asarray(res.results[c]["o"], dtype=np.float32)
        for s in range(NSLOT):
            j, r, _ = _slot_item(s, team, g)
            C[r * P:(r + 1) * P, j * CW:(j + 1) * CW] = o[s]
    if trace:
        kernel.last_exec_time_ns = res.exec_time_ns
        kernel.last_results = res
    return C
